# revision 8
# baseline (speedup 1.0000x reference)
"""Trainium2 Bass kernel for nn_CCL__69277822485245 (spectral conv via DCT/FFT).

Math: the reference's rFFT along W cancels into a circular 5-tap convolution,
and the DCT-II sandwich M @ diag(D[:,s]) @ D collapses into 5 dense 128x128
matrices G_s (precomputed on host). Per batch element:

    u_s[i, m, w] = sum_h G_s[m, h] x[i, h, w]                  (stage 1)
    out[o, m, n] = sum_{s,t,i} W[o,i,s,t] u_s[i, m, (n-t)%W] + bias[o]

Sharding: data-parallel over batch B=8 across the 8 NeuronCores (1 each).

v2 layout — w-parity packing (no duplication, no w-halo in stage 1):
  stage 1: lhsT = x2[h=128, (w-pair jp -> 128 cols: w=2jp i0..63, w=2jp+1
      i0..63)] (stationary, one load per jp), rhs = gt[h, (mh, s, m)] N=320.
      psum[(wp,i), (s,m)] -> one straight (non-transposing) copy per (jp,mh)
      into u[(wp,i), s, HALO+jp, m]; jp 62,63 also copied to the front halo
      slots (circular W).
  stage 2: output n split by parity p; kernel taps t pair across partition
      halves by w-parity of n-t. Per (s,p): two K=128 pairs + one K=64 solo,
      each a jp-offset slice of u. 15 accumulating matmuls per psum chunk,
      chunk = [o=128, (jp=64, m=8)] so finished output is contiguous per
      m-row -> efficient streaming DMA out per 8-m block.

DTYPE "bf16": 1 cyc/row matmuls, rel err ~ 3e-3 (gate 2e-2).
"""

import numpy as np

H = 128
W = 128
CI = 64
CO = 128
KH = 5
KW = 5
B = 8

MH = 64          # m-half processed per outer iteration
JP = W // 2      # 64 w-pairs
HALO = 2         # front jp-halo (circular W wrap for t-shifts)
JX = HALO + JP   # 66

DTYPE = "bf16"

_PROG = None
_CONSTS = None
_RUN_OPTS = {}     # test harness may set e.g. {"trace": True, "trace_cores": [0]}
_LAST_RESULT = None

# stage-2 slot groups per parity: (s, gi) -> (jp_offset, kbase, kk)
#   p=0: gi0 = (t2|t1) off -1, gi1 = (t4|t3) off -2, gi2 = (t0|--) off 0 K=64 lo
#   p=1: gi0 = (t1|t0) off  0, gi1 = (t3|t2) off -1, gi2 = (--|t4) off -2 K=64 hi
_GROUPS = {
    0: [(-1, 0, 128), (-2, 0, 128), (0, 0, 64)],
    1: [(0, 0, 128), (-1, 0, 128), (-2, 64, 64)],
}


def _np_dt():
    if DTYPE == "bf16":
        import ml_dtypes
        return ml_dtypes.bfloat16
    return np.float32


def _build_consts():
    n = np.arange(H, dtype=np.float64)
    ang = np.pi * (2.0 * n[None, :] + 1.0) * n[:, None] / (2.0 * H)  # [k, h]
    D = 2.0 * np.cos(ang)
    wgt = np.where(n == 0, 0.5, 1.0)
    M = (np.cos(ang).T * wgt[None, :]) / (2.0 * H)                    # [m, k]
    G = np.stack([M @ (D[:, s:s + 1] * D) for s in range(KH)])        # [s, m, h]
    # gt layout [h, (mh, s, m)]: col = mh*320 + s*64 + ml
    GT = (G.transpose(2, 0, 1)                # [h, s, m]
            .reshape(H, KH, 2, MH)            # [h, s, mh, ml]
            .transpose(0, 2, 1, 3)            # [h, mh, s, ml]
            .reshape(H, KH * H))
    return np.ascontiguousarray(GT).astype(_np_dt())


def _build_wstack(weight):
    # wst[(d,i), (p, s, gi, o)]: see _GROUPS; d = w-parity partition half
    wst = np.zeros((128, 2 * KH * 3 * CO), np.float32)
    col = 0
    for p in range(2):
        for s in range(KH):
            Wl = weight[:, :, s, :]          # [o, i, t]
            if p == 0:
                pairs = [(2, 1), (4, 3)]     # (lower half t, upper half t)
                solo = (0, 0)                # (t, kbase)
            else:
                pairs = [(1, 0), (3, 2)]
                solo = (4, 64)
            for tl, tu in pairs:
                wst[0:64, col:col + CO] = Wl[:, :, tl].T
                wst[64:128, col:col + CO] = Wl[:, :, tu].T
                col += CO
            t, kb = solo
            wst[kb:kb + 64, col:col + CO] = Wl[:, :, t].T
            col += CO
    return np.ascontiguousarray(wst).astype(_np_dt())


def _build_program():
    import concourse.mybir as mybir
    import concourse.tile as tile
    from concourse import bacc

    f32 = mybir.dt.float32
    mmdt = {"bf16": mybir.dt.bfloat16,
            "f32r": mybir.dt.float32r,
            "f32": mybir.dt.float32}[DTYPE]

    nc = bacc.Bacc("TRN2", target_bir_lowering=False, debug=False,
                   enable_asserts=False, num_devices=B)
    x_d = nc.dram_tensor("x", [H, W * CI], mmdt, kind="ExternalInput").ap()
    g_d = nc.dram_tensor("g", [H, KH * H], mmdt, kind="ExternalInput").ap()
    w_d = nc.dram_tensor("wt", [128, 2 * KH * 3 * CO], mmdt,
                         kind="ExternalInput").ap()
    b_d = nc.dram_tensor("bias", [CO, 1], f32, kind="ExternalInput").ap()
    o_d = nc.dram_tensor("out", [CO, H, W], f32, kind="ExternalOutput").ap()

    with tile.TileContext(nc) as tc:
        with (
            tc.tile_pool(name="const", bufs=1) as cpool,
            tc.tile_pool(name="u", bufs=1) as upool,
            tc.tile_pool(name="oacc", bufs=1) as opool,
            tc.tile_pool(name="ps1", bufs=2, space="PSUM") as ps1,
            tc.tile_pool(name="ps2", bufs=2, space="PSUM") as ps2,
        ):
            xt = cpool.tile([H, W * CI], mmdt)
            # first w-quarter of x lands first so stage 1 starts early
            nc.sync.dma_start(xt[:, 0:2048], x_d[:, 0:2048])
            gt = cpool.tile([H, KH * H], mmdt)
            nc.sync.dma_start(gt[:], g_d)
            wt = cpool.tile([128, 2 * KH * 3 * CO], mmdt)
            nc.sync.dma_start(wt[:], w_d)
            bt = cpool.tile([CO, 1], f32)
            nc.sync.dma_start(bt[:], b_d)
            for c in range(1, 4):
                nc.sync.dma_start(xt[:, c * 2048:(c + 1) * 2048],
                                  x_d[:, c * 2048:(c + 1) * 2048])

            import concourse.mybir as _mb

            def mm(out, lhsT, rhs, start, stop, reload):
                inst = nc.tensor.matmul(out, lhsT, rhs, start=start, stop=stop)
                if not reload:      # stationary weights already in the array
                    inst.ldweights = False

            def stage1():
                us = []
                for mh in range(2):
                    u = upool.tile([128, KH * JX * MH], mmdt, tag=f"u{mh}")
                    us.append(u[:].rearrange("p (s j m) -> p s j m",
                                             s=KH, j=JX))
                for jp in range(JP):
                    lhsT = xt[:, jp * 128:(jp + 1) * 128]
                    for mh in range(2):
                        p1 = ps1.tile([128, KH * MH], f32)
                        mm(p1[:], lhsT,
                           gt[:, mh * KH * MH:(mh + 1) * KH * MH],
                           start=True, stop=True, reload=(mh == 0))
                        pv = p1[:].rearrange("p (s m) -> p s m", s=KH)
                        eng = nc.vector if mh == 0 else nc.scalar
                        if mh == 0:
                            eng.tensor_copy(us[mh][:, :, HALO + jp, :], pv)
                        else:
                            eng.activation(
                                us[mh][:, :, HALO + jp, :], pv,
                                _mb.ActivationFunctionType.Identity)
                        if jp >= JP - HALO:   # circular wrap into front halo
                            nc.vector.tensor_copy(
                                us[mh][:, :, jp - (JP - HALO), :], pv)
                return us

            def stage2(u4, mh):
                oacc = opool.tile([CO, MH * W], f32, tag=f"oacc{mh}")
                # [o, m, jn-pair, parity]
                oa4 = oacc[:].rearrange("p (m j q) -> p m j q", m=MH, q=2)
                for p in range(2):
                    for sweep in ((0, 1, 2), (3, 4, 5), (6, 7)):
                        p2s = [ps2.tile([128, JP * 8], f32, tag=f"p2_{k}",
                                        name=f"p2_{k}")
                               for k in range(len(sweep))]
                        for gi in range(3 * KH):
                            s, g = divmod(gi, 3)
                            off, kb, kk = _GROUPS[p][g]
                            gb = p * 15 + s * 3 + g
                            lhsT = wt[kb:kb + kk, gb * CO:(gb + 1) * CO]
                            for k, mc in enumerate(sweep):
                                rhs = u4[kb:kb + kk, s,
                                         HALO + off:HALO + off + JP,
                                         mc * 8:(mc + 1) * 8]
                                mm(p2s[k][:], lhsT, rhs, start=(gi == 0),
                                   stop=(gi == 3 * KH - 1), reload=(k == 0))
                        for k, mc in enumerate(sweep):
                            p23 = p2s[k][:].rearrange("p (j m) -> p j m", j=JP)
                            nc.scalar.activation(
                                oa4[:, mc * 8:(mc + 1) * 8, :, p],
                                p23[:].transpose([0, 2, 1]),
                                _mb.ActivationFunctionType.Identity,
                                bias=bt[:])
                            if p == 1:   # both parities done -> stream out
                                nc.sync.dma_start(
                                    o_d[:, mh * MH + mc * 8:
                                        mh * MH + (mc + 1) * 8, :],
                                    oacc[:, mc * 8 * W:(mc + 1) * 8 * W])

            u0, u1 = stage1()
            stage2(u0, 0)
            stage2(u1, 1)
    nc.compile()
    return nc


def _get_prog():
    global _PROG
    if _PROG is None:
        _PROG = _build_program()
    return _PROG


def kernel(x, weight, bias):
    from concourse.bass_utils import run_bass_kernel_spmd

    global _CONSTS
    if _CONSTS is None:
        _CONSTS = _build_consts()
    GT = _CONSTS

    x = np.ascontiguousarray(np.asarray(x, dtype=np.float32))
    weight = np.ascontiguousarray(np.asarray(weight, dtype=np.float32))
    bias = np.ascontiguousarray(np.asarray(bias, dtype=np.float32))

    wst = _build_wstack(weight)
    b2 = np.ascontiguousarray(bias.reshape(CO, 1))

    in_maps = []
    for b in range(B):
        # x2[h, (w, i)]
        x2 = np.ascontiguousarray(
            x[b].transpose(1, 2, 0).reshape(H, W * CI)).astype(_np_dt())
        in_maps.append({"x": x2, "g": GT, "wt": wst, "bias": b2})

    res = run_bass_kernel_spmd(_get_prog(), in_maps, core_ids=list(range(B)),
                               **_RUN_OPTS)
    global _LAST_RESULT
    _LAST_RESULT = res
    out = np.stack([res.results[b]["out"] for b in range(B)], axis=0)
    return np.ascontiguousarray(out.astype(np.float32))


# revision 11
# speedup vs baseline: 1.0280x; 1.0280x over previous
"""Trainium2 Bass kernel for nn_CCL__69277822485245 (spectral conv via DCT/FFT).

Math: the reference's rFFT along W cancels into a circular 5-tap convolution,
and the DCT-II sandwich M @ diag(D[:,s]) @ D collapses into 5 dense 128x128
matrices G_s (precomputed on host). Per batch element:

    u_s[i, m, w] = sum_h G_s[m, h] x[i, h, w]                  (stage 1)
    out[o, m, n] = sum_{s,t,i} W[o,i,s,t] u_s[i, m, (n-t)%W] + bias[o]

Sharding: data-parallel over batch B=8 across the 8 NeuronCores (1 each).

v2 layout — w-parity packing (no duplication, no w-halo in stage 1):
  stage 1: lhsT = x2[h=128, (w-pair jp -> 128 cols: w=2jp i0..63, w=2jp+1
      i0..63)] (stationary, one load per jp), rhs = gt[h, (mh, s, m)] N=320.
      psum[(wp,i), (s,m)] -> one straight (non-transposing) copy per (jp,mh)
      into u[(wp,i), s, HALO+jp, m]; jp 62,63 also copied to the front halo
      slots (circular W).
  stage 2: output n split by parity p; kernel taps t pair across partition
      halves by w-parity of n-t. Per (s,p): two K=128 pairs + one K=64 solo,
      each a jp-offset slice of u. 15 accumulating matmuls per psum chunk,
      chunk = [o=128, (jp=64, m=8)] so finished output is contiguous per
      m-row -> efficient streaming DMA out per 8-m block.

DTYPE "bf16": 1 cyc/row matmuls, rel err ~ 3e-3 (gate 2e-2).
"""

import numpy as np

H = 128
W = 128
CI = 64
CO = 128
KH = 5
KW = 5
B = 8

MH = 64          # m-half processed per outer iteration
JP = W // 2      # 64 w-pairs
HALO = 2         # front jp-halo (circular W wrap for t-shifts)
JX = HALO + JP   # 66

DTYPE = "bf16"

_PROG = None
_CONSTS = None
_RUN_OPTS = {}     # test harness may set e.g. {"trace": True, "trace_cores": [0]}
_LAST_RESULT = None

# stage-2 slot groups per parity: (s, gi) -> (jp_offset, kbase, kk)
#   p=0: gi0 = (t2|t1) off -1, gi1 = (t4|t3) off -2, gi2 = (t0|--) off 0 K=64 lo
#   p=1: gi0 = (t1|t0) off  0, gi1 = (t3|t2) off -1, gi2 = (--|t4) off -2 K=64 hi
_GROUPS = {
    0: [(-1, 0, 128), (-2, 0, 128), (0, 0, 64)],
    1: [(0, 0, 128), (-1, 0, 128), (-2, 64, 64)],
}


def _np_dt():
    if DTYPE == "bf16":
        import ml_dtypes
        return ml_dtypes.bfloat16
    return np.float32


def _build_consts():
    n = np.arange(H, dtype=np.float64)
    ang = np.pi * (2.0 * n[None, :] + 1.0) * n[:, None] / (2.0 * H)  # [k, h]
    D = 2.0 * np.cos(ang)
    wgt = np.where(n == 0, 0.5, 1.0)
    M = (np.cos(ang).T * wgt[None, :]) / (2.0 * H)                    # [m, k]
    G = np.stack([M @ (D[:, s:s + 1] * D) for s in range(KH)])        # [s, m, h]
    # gt layout [h, (mh, s, m)]: col = mh*320 + s*64 + ml
    GT = (G.transpose(2, 0, 1)                # [h, s, m]
            .reshape(H, KH, 2, MH)            # [h, s, mh, ml]
            .transpose(0, 2, 1, 3)            # [h, mh, s, ml]
            .reshape(H, KH * H))
    return np.ascontiguousarray(GT).astype(_np_dt())


def _build_wstack(weight):
    # wst[(d,i), (p, s, gi, o)]: see _GROUPS; d = w-parity partition half
    wst = np.zeros((128, 2 * KH * 3 * CO), np.float32)
    col = 0
    for p in range(2):
        for s in range(KH):
            Wl = weight[:, :, s, :]          # [o, i, t]
            if p == 0:
                pairs = [(2, 1), (4, 3)]     # (lower half t, upper half t)
                solo = (0, 0)                # (t, kbase)
            else:
                pairs = [(1, 0), (3, 2)]
                solo = (4, 64)
            for tl, tu in pairs:
                wst[0:64, col:col + CO] = Wl[:, :, tl].T
                wst[64:128, col:col + CO] = Wl[:, :, tu].T
                col += CO
            t, kb = solo
            wst[kb:kb + 64, col:col + CO] = Wl[:, :, t].T
            col += CO
    return np.ascontiguousarray(wst).astype(_np_dt())


def _build_program():
    import concourse.mybir as mybir
    import concourse.tile as tile
    from concourse import bacc

    f32 = mybir.dt.float32
    mmdt = {"bf16": mybir.dt.bfloat16,
            "f32r": mybir.dt.float32r,
            "f32": mybir.dt.float32}[DTYPE]

    nc = bacc.Bacc("TRN2", target_bir_lowering=False, debug=False,
                   enable_asserts=False, num_devices=B)
    x_d = nc.dram_tensor("x", [H, W * CI], mmdt, kind="ExternalInput").ap()
    g_d = nc.dram_tensor("g", [H, KH * H], mmdt, kind="ExternalInput").ap()
    w_d = nc.dram_tensor("wt", [128, 2 * KH * 3 * CO], mmdt,
                         kind="ExternalInput").ap()
    b_d = nc.dram_tensor("bias", [CO, 1], f32, kind="ExternalInput").ap()
    o_d = nc.dram_tensor("out", [CO, H, W], f32, kind="ExternalOutput").ap()

    with tile.TileContext(nc) as tc:
        with (
            tc.tile_pool(name="const", bufs=1) as cpool,
            tc.tile_pool(name="u", bufs=1) as upool,
            tc.tile_pool(name="oacc", bufs=1) as opool,
            tc.tile_pool(name="ps1", bufs=2, space="PSUM") as ps1,
            tc.tile_pool(name="ps2", bufs=2, space="PSUM") as ps2,
        ):
            xt = cpool.tile([H, W * CI], mmdt)
            # first w-quarter of x lands first so stage 1 starts early
            nc.sync.dma_start(xt[:, 0:2048], x_d[:, 0:2048])
            gt = cpool.tile([H, KH * H], mmdt)
            nc.sync.dma_start(gt[:], g_d)
            wt = cpool.tile([128, 2 * KH * 3 * CO], mmdt)
            nc.sync.dma_start(wt[:], w_d)
            bt = cpool.tile([CO, 1], f32)
            nc.sync.dma_start(bt[:], b_d)
            for c in range(1, 4):
                nc.sync.dma_start(xt[:, c * 2048:(c + 1) * 2048],
                                  x_d[:, c * 2048:(c + 1) * 2048])

            import concourse.mybir as _mb

            def mm(out, lhsT, rhs, start, stop, reload):
                inst = nc.tensor.matmul(out, lhsT, rhs, start=start, stop=stop)
                if not reload:      # stationary weights already in the array
                    inst.ldweights = False

            def stage1():
                u = upool.tile([128, 2 * KH * JX * MH], mmdt)
                u5 = u[:].rearrange("p (h s j m) -> p h s j m",
                                    h=2, s=KH, j=JX)
                for jp in range(JP):
                    lhsT = xt[:, jp * 128:(jp + 1) * 128]
                    # mh halves at 512-col (bank) offsets so neither matmul
                    # crosses a PSUM bank boundary
                    p1 = ps1.tile([128, 1024], f32)
                    for mh in range(2):
                        mm(p1[:, mh * 512:mh * 512 + KH * MH], lhsT,
                           gt[:, mh * KH * MH:(mh + 1) * KH * MH],
                           start=True, stop=True, reload=(mh == 0))
                    pv = (p1[:].rearrange("p (h q) -> p h q", h=2)[:, :, 0:KH * MH]
                          .rearrange("p h (s m) -> p h s m", s=KH))
                    if jp % 2 == 0:
                        nc.vector.tensor_copy(u5[:, :, :, HALO + jp, :], pv)
                    else:
                        nc.scalar.activation(
                            u5[:, :, :, HALO + jp, :], pv,
                            _mb.ActivationFunctionType.Identity)
                    if jp >= JP - HALO:   # circular wrap into front halo
                        nc.vector.tensor_copy(
                            u5[:, :, :, jp - (JP - HALO), :], pv)
                return (u5[:, 0], u5[:, 1])

            def stage2(u4, mh):
                oacc = opool.tile([CO, MH * W], f32, tag=f"oacc{mh}")
                # [o, m, jn-pair, parity]
                oa4 = oacc[:].rearrange("p (m j q) -> p m j q", m=MH, q=2)
                for p in range(2):
                    for sweep in ((0, 1), (2, 3), (4, 5), (6, 7)):
                        p2s = [ps2.tile([128, JP * 8], f32, tag=f"p2_{k}",
                                        name=f"p2_{k}")
                               for k in range(len(sweep))]
                        for gi in range(3 * KH):
                            s, g = divmod(gi, 3)
                            off, kb, kk = _GROUPS[p][g]
                            gb = p * 15 + s * 3 + g
                            lhsT = wt[kb:kb + kk, gb * CO:(gb + 1) * CO]
                            for k, mc in enumerate(sweep):
                                rhs = u4[kb:kb + kk, s,
                                         HALO + off:HALO + off + JP,
                                         mc * 8:(mc + 1) * 8]
                                mm(p2s[k][:], lhsT, rhs, start=(gi == 0),
                                   stop=(gi == 3 * KH - 1), reload=(k == 0))
                        for k, mc in enumerate(sweep):
                            p23 = p2s[k][:].rearrange("p (j m) -> p j m", j=JP)
                            nc.scalar.activation(
                                oa4[:, mc * 8:(mc + 1) * 8, :, p],
                                p23[:].transpose([0, 2, 1]),
                                _mb.ActivationFunctionType.Identity,
                                bias=bt[:])
                            if p == 1:   # both parities done -> stream out
                                nc.sync.dma_start(
                                    o_d[:, mh * MH + mc * 8:
                                        mh * MH + (mc + 1) * 8, :],
                                    oacc[:, mc * 8 * W:(mc + 1) * 8 * W])

            u0, u1 = stage1()
            stage2(u0, 0)
            stage2(u1, 1)
    nc.compile()
    return nc


def _get_prog():
    global _PROG
    if _PROG is None:
        _PROG = _build_program()
    return _PROG


def kernel(x, weight, bias):
    from concourse.bass_utils import run_bass_kernel_spmd

    global _CONSTS
    if _CONSTS is None:
        _CONSTS = _build_consts()
    GT = _CONSTS

    x = np.ascontiguousarray(np.asarray(x, dtype=np.float32))
    weight = np.ascontiguousarray(np.asarray(weight, dtype=np.float32))
    bias = np.ascontiguousarray(np.asarray(bias, dtype=np.float32))

    wst = _build_wstack(weight)
    b2 = np.ascontiguousarray(bias.reshape(CO, 1))

    in_maps = []
    for b in range(B):
        # x2[h, (w, i)]
        x2 = np.ascontiguousarray(
            x[b].transpose(1, 2, 0).reshape(H, W * CI)).astype(_np_dt())
        in_maps.append({"x": x2, "g": GT, "wt": wst, "bias": b2})

    res = run_bass_kernel_spmd(_get_prog(), in_maps, core_ids=list(range(B)),
                               **_RUN_OPTS)
    global _LAST_RESULT
    _LAST_RESULT = res
    out = np.stack([res.results[b]["out"] for b in range(B)], axis=0)
    return np.ascontiguousarray(out.astype(np.float32))
